# revision 3
# baseline (speedup 1.0000x reference)
"""CrossViewFusion Trainium2 kernel (v2).

Math (per batch row b):
  seq = [x_cc; x_mlo]                  # 2 views, D=512 each
  qkv = seq @ in_proj_w.T + b          # per view: q,k,v (512 each, 8 heads x 64)
  scores[h,qv] . keys -> softmax over 2 keys (mlo maskable) == sigmoid fold:
    w_view = sigmoid(q_view . (k_cc - k_mlo) / 8)   (per head)
  ao_view = v_mlo + w_view *_head (v_cc - v_mlo)
  h_view = x_view + ao_view @ out_w.T + out_b
  t = LN(h); g = sigmoid([t_cc; t_mlo] @ gate_w.T + gate_b)
  out = has_mlo ? g*t_cc + (1-g)*t_mlo : t_cc

v2 design (vs v1 baseline):
  - difference trick: dk = dx@Wk.T, dv = dx@Wv.T with dx = x_cc - x_mlo
    (host-computed); v_mlo@Wo.T folded host-side into A' = x_mlo@(Wv.T Wo.T + I)
    so h_mlo = A' + B_mlo, h_cc = A' + dx + B_cc with B = (w*dv)@Wo.T.
    10 -> 9 DxD matmul units per row.
  - fp8(e4m3) DoubleRow matmuls for the score path (q_cc, q_mlo, dk): PE
    double-pumped, 9 -> 7.5 effective units. v/out/gate stay bf16 (accuracy).
  - all input-side transposes + mask scalar algebra precomputed on host;
    on-chip DMA transposes only for wdv and t (4 per super-group).
  - PSUM-resident consumption (no qkv drains); LN via bn_stats/bn_aggr.
  - 3-stage software-pipelined emission (front / proj / gate) so the PE queue
    never stalls: keeps the PE p-state at 2.4 GHz.
"""

import sys

for _p in ("/opt/trn_rl_repo",):
    if _p not in sys.path:
        sys.path.append(_p)

import numpy as np
import ml_dtypes

B, D, H, HD = 65536, 512, 8, 64
NCORES = 8
BS = B // NCORES          # rows per core
P = 128                   # SBUF partitions
SG = 2                    # row-groups of 128 per super-group
EPS = 1e-5

BF16 = ml_dtypes.bfloat16
FP8 = ml_dtypes.float8_e4m3

_cache = {}

# engine placement knobs
DRAIN_SCORES = True   # ACT-drain q/dk/dv psum to bf16 so DVE runs 2x/4x modes


def _build(flags, bs=BS):
    """flags = (zero_qb, zero_hb, zero_gb2, unit_lng, zero_lnb)."""
    import concourse.mybir as mybir
    from concourse import bacc, tile
    from concourse.bass import ts
    from contextlib import ExitStack

    zero_qb, zero_hb, zero_gb2, unit_lng, zero_lnb = flags
    nsg = bs // (P * SG)
    W2 = SG * P                # rows per super-group (256)
    f32 = mybir.dt.float32
    bf16 = mybir.dt.bfloat16
    f8 = mybir.dt.float8e4
    AF = mybir.ActivationFunctionType
    OP = mybir.AluOpType
    AX = mybir.AxisListType
    DR = mybir.MatmulPerfMode.DoubleRow

    nc = bacc.Bacc("TRN2", target_bir_lowering=False, debug=False,
                   enable_asserts=False)

    # ---- DRAM I/O (host-prepped layouts) ----
    xT8c_d = nc.dram_tensor("xT8c", [D, bs], f8, kind="ExternalInput").ap()
    xT8m_d = nc.dram_tensor("xT8m", [D, bs], f8, kind="ExternalInput").ap()
    dxT8_d = nc.dram_tensor("dxT8", [D, bs], f8, kind="ExternalInput").ap()
    xTm_d = nc.dram_tensor("xTm", [D, bs], bf16, kind="ExternalInput").ap()
    dxT_d = nc.dram_tensor("dxT", [D, bs], bf16, kind="ExternalInput").ap()
    dxF_d = nc.dram_tensor("dxF", [bs, D], bf16, kind="ExternalInput").ap()
    mc_d = nc.dram_tensor("mc", [bs, 6], f32, kind="ExternalInput").ap()
    wqk8_d = nc.dram_tensor("wqk8", [D, 2 * D], f8, kind="ExternalInput").ap()
    wvT_d = nc.dram_tensor("wvT", [D, D], bf16, kind="ExternalInput").ap()
    wvoiT_d = nc.dram_tensor("wvoiT", [D, D], bf16, kind="ExternalInput").ap()
    owT_d = nc.dram_tensor("owT", [D, D], bf16, kind="ExternalInput").ap()
    gwT_d = nc.dram_tensor("gwT", [2 * D, D], bf16, kind="ExternalInput").ap()
    opt_in = {}
    if not zero_qb:
        opt_in["qb2"] = nc.dram_tensor("qb2", [D], f32, kind="ExternalInput").ap()
    if not zero_hb:
        opt_in["hb"] = nc.dram_tensor("hb", [D], f32, kind="ExternalInput").ap()
    if not zero_gb2:
        opt_in["gb2"] = nc.dram_tensor("gb2", [D], f32, kind="ExternalInput").ap()
    if not unit_lng:
        opt_in["lng"] = nc.dram_tensor("lng", [D], f32, kind="ExternalInput").ap()
    if not zero_lnb:
        opt_in["lnb"] = nc.dram_tensor("lnb", [D], f32, kind="ExternalInput").ap()
    out_d = nc.dram_tensor("out", [bs, D], f32, kind="ExternalOutput").ap()

    with tile.TileContext(nc) as tc, ExitStack() as ctx:
        wpool = ctx.enter_context(tc.tile_pool(name="wpool", bufs=1))
        ld = ctx.enter_context(tc.tile_pool(name="ld", bufs=2))
        sc = ctx.enter_context(tc.tile_pool(name="sc", bufs=2))
        mid = ctx.enter_context(tc.tile_pool(name="mid", bufs=3))
        bstp = ctx.enter_context(tc.tile_pool(name="bstp", bufs=2))
        tpool = ctx.enter_context(tc.tile_pool(name="tpool", bufs=3))
        gp = ctx.enter_context(tc.tile_pool(name="gp", bufs=2))
        mcp = ctx.enter_context(tc.tile_pool(name="mcp", bufs=4))
        ps_trio = ctx.enter_context(tc.tile_pool(name="ps_trio", bufs=3, space="PSUM"))
        ps_dva = ctx.enter_context(tc.tile_pool(name="ps_dva", bufs=2, space="PSUM"))
        ps_b = ctx.enter_context(tc.tile_pool(name="ps_b", bufs=2, space="PSUM"))
        ps_pg = ctx.enter_context(tc.tile_pool(name="ps_pg", bufs=1, space="PSUM"))

        # ---- resident weights ----
        wqk8_sb = wpool.tile([P, 4, 2 * D], f8)
        nc.sync.dma_start(wqk8_sb[:], wqk8_d.rearrange("(c p) f -> p c f", p=P))
        wvT_sb = wpool.tile([P, 4, D], bf16)
        nc.sync.dma_start(wvT_sb[:], wvT_d.rearrange("(c p) f -> p c f", p=P))
        wvoiT_sb = wpool.tile([P, 4, D], bf16)
        nc.sync.dma_start(wvoiT_sb[:], wvoiT_d.rearrange("(c p) f -> p c f", p=P))
        owT_sb = wpool.tile([P, 4, D], bf16)
        nc.sync.dma_start(owT_sb[:], owT_d.rearrange("(c p) f -> p c f", p=P))
        gwT_sb = wpool.tile([P, 8, D], bf16)
        nc.sync.dma_start(gwT_sb[:], gwT_d.rearrange("(c p) f -> p c f", p=P))

        def bcast_tile(name, dram_ap, n):
            t32 = wpool.tile([P, n], f32, name=name)
            nc.sync.dma_start(t32[:], dram_ap[None, :].to_broadcast((P, n)))
            return t32

        qb2_bc = None if zero_qb else bcast_tile("qb2_bc", opt_in["qb2"], D)
        hb_bc = None if zero_hb else bcast_tile("hb_bc", opt_in["hb"], D)
        gb2_bc = None if zero_gb2 else bcast_tile("gb2_bc", opt_in["gb2"], D)
        lng_bc = None if unit_lng else bcast_tile("lng_bc", opt_in["lng"], D)
        lnb_bc = None if zero_lnb else bcast_tile("lnb_bc", opt_in["lnb"], D)

        eps_p1 = wpool.tile([P, 1], f32)
        nc.vector.memset(eps_p1[:], EPS)

        # PE warmup: keep the array clocked while weight DMAs land.
        wu_s = wpool.tile([P, D], bf16)
        nc.vector.memset(wu_s[:], 0)
        wu_ps = ps_pg.tile([P, D], f32, name="wu_ps", tag="pg")
        for _ in range(12):
            nc.tensor.matmul(wu_ps[:], wu_s[:, 0:P], wu_s[:],
                             start=True, stop=True)

        state = {}

        def stage_front(s):
            rows = ts(s, W2)
            st = {}
            # loads (host-transposed: [D, bs] -> [p, c, r])
            xT8c = ld.tile([P, 4, W2], f8, name="xT8c")
            nc.sync.dma_start(xT8c[:], xT8c_d[:, rows].rearrange(
                "(c p) r -> p c r", p=P))
            xT8m = ld.tile([P, 4, W2], f8, name="xT8m")
            nc.sync.dma_start(xT8m[:], xT8m_d[:, rows].rearrange(
                "(c p) r -> p c r", p=P))
            dxT8 = ld.tile([P, 4, W2], f8, name="dxT8")
            nc.sync.dma_start(dxT8[:], dxT8_d[:, rows].rearrange(
                "(c p) r -> p c r", p=P))
            xTm = ld.tile([P, 4, W2], bf16, name="xTm")
            nc.sync.dma_start(xTm[:], xTm_d[:, rows].rearrange(
                "(c p) r -> p c r", p=P))
            dxT = ld.tile([P, 4, W2], bf16, name="dxT")
            nc.sync.dma_start(dxT[:], dxT_d[:, rows].rearrange(
                "(c p) r -> p c r", p=P))
            dxF = mid.tile([P, SG, D], bf16, name="dxF")
            nc.sync.dma_start(dxF[:], dxF_d[rows, :].rearrange(
                "(n p) d -> p n d", p=P))
            mc = mcp.tile([P, SG, 6], f32, name="mc")
            nc.sync.dma_start(mc[:], mc_d[rows, :].rearrange(
                "(n p) c -> p n c", p=P))
            st["mc"] = mc
            st["dxF"] = dxF

            q_sb = sc.tile([P, SG, 2, D], bf16, name="q_sb")
            dv_sb = sc.tile([P, SG, D], bf16, name="dv_sb")
            pcc = sc.tile([P, SG, D], bf16, name="pcc")
            pmlo = sc.tile([P, SG, D], bf16, name="pmlo")
            sAll = sc.tile([P, SG, 2, H], f32, name="sAll")
            a_sb = mid.tile([P, SG, D], bf16, name="a_sb")
            hc0 = mid.tile([P, SG, D], bf16, name="hc0")
            wdvB = mid.tile([P, 2, SG, D], bf16, name="wdvB")

            for n in range(SG):
                rr = ts(n, P)
                # --- score-path fp8 DoubleRow matmuls ---
                qcc_ps = ps_trio.tile([P, D], f32, name="qcc_ps", tag="trio")
                for cp in range(2):
                    nc.tensor.matmul(qcc_ps[:], xT8c[:, 2 * cp:2 * cp + 2, rr],
                                     wqk8_sb[:, 2 * cp:2 * cp + 2, 0:D],
                                     start=(cp == 0), stop=(cp == 1),
                                     perf_mode=DR)
                qml_ps = ps_trio.tile([P, D], f32, name="qml_ps", tag="trio")
                for cp in range(2):
                    nc.tensor.matmul(qml_ps[:], xT8m[:, 2 * cp:2 * cp + 2, rr],
                                     wqk8_sb[:, 2 * cp:2 * cp + 2, 0:D],
                                     start=(cp == 0), stop=(cp == 1),
                                     perf_mode=DR)
                dk_ps = ps_trio.tile([P, D], f32, name="dk_ps", tag="trio")
                for cp in range(2):
                    nc.tensor.matmul(dk_ps[:], dxT8[:, 2 * cp:2 * cp + 2, rr],
                                     wqk8_sb[:, 2 * cp:2 * cp + 2, D:2 * D],
                                     start=(cp == 0), stop=(cp == 1),
                                     perf_mode=DR)
                if not zero_qb:
                    nc.vector.tensor_add(qcc_ps[:], qcc_ps[:], qb2_bc[:])
                    nc.vector.tensor_add(qml_ps[:], qml_ps[:], qb2_bc[:])
                # --- bf16 matmuls: dv, A' ---
                dv_ps = ps_dva.tile([P, D], f32, name="dv_ps", tag="dva")
                for c in range(4):
                    nc.tensor.matmul(dv_ps[:], dxT[:, c, rr], wvT_sb[:, c, :],
                                     start=(c == 0), stop=(c == 3))
                ap_ps = ps_dva.tile([P, D], f32, name="ap_ps", tag="dva")
                for c in range(4):
                    nc.tensor.matmul(ap_ps[:], xTm[:, c, rr], wvoiT_sb[:, c, :],
                                     start=(c == 0), stop=(c == 3))

                # scores: products (+ drains for DVE fast modes)
                if DRAIN_SCORES:
                    nc.scalar.copy(q_sb[:, n, 0, :], qcc_ps[:])
                    nc.scalar.copy(q_sb[:, n, 1, :], qml_ps[:])
                    dkv = sc.tile([P, D], bf16, name="dk_sb", bufs=2)
                    nc.scalar.copy(dkv[:], dk_ps[:])
                    nc.vector.tensor_mul(pcc[:, n, :], q_sb[:, n, 0, :], dkv[:])
                    nc.vector.tensor_mul(pmlo[:, n, :], q_sb[:, n, 1, :], dkv[:])
                else:
                    nc.vector.tensor_mul(pcc[:, n, :], qcc_ps[:], dk_ps[:])
                    nc.vector.tensor_mul(pmlo[:, n, :], qml_ps[:], dk_ps[:])
                nc.vector.reduce_sum(
                    sAll[:, n, 0, :],
                    pcc[:, n, :].rearrange("p (h e) -> p h e", e=HD), axis=AX.X)
                nc.vector.reduce_sum(
                    sAll[:, n, 1, :],
                    pmlo[:, n, :].rearrange("p (h e) -> p h e", e=HD), axis=AX.X)

                # A' drain (+ optional hbias)
                if not zero_hb:
                    nc.vector.tensor_add(ap_ps[:], ap_ps[:], hb_bc[:])
                nc.scalar.copy(a_sb[:, n, :], ap_ps[:])
                # dv drain
                if DRAIN_SCORES:
                    nc.scalar.copy(dv_sb[:, n, :], dv_ps[:])

                # hc0 = A' + dx (cc residual partial)
                nc.vector.tensor_add(hc0[:, n, :], a_sb[:, n, :], dxF[:, n, :])

                # w = sigmoid(s/8), folded mask -> per (vi) weights
                wsig = sc.tile([P, 2, H], bf16, name="wsig", bufs=2)
                nc.scalar.activation(
                    wsig[:].rearrange("p a h -> p (a h)"),
                    sAll[:, n, :, :].rearrange("p a h -> p (a h)"),
                    AF.Sigmoid, scale=1.0 / np.sqrt(HD))
                weff = sc.tile([P, 2, H], bf16, name="weff", bufs=2)
                nc.vector.tensor_scalar(weff[:], wsig[:], mc[:, n, 0:1],
                                        mc[:, n, 1:2], op0=OP.mult, op1=OP.add)
                # wdv = weff *_head dv
                for vi in range(2):
                    src = dv_sb[:, n, :] if DRAIN_SCORES else dv_ps[:]
                    nc.vector.tensor_mul(
                        wdvB[:, vi, n, :].rearrange("p (h e) -> p h e", e=HD),
                        src.rearrange("p (h e) -> p h e", e=HD),
                        weff[:, vi, :].unsqueeze(2).broadcast_to((P, H, HD)))

            wdvT = mid.tile([P, 2, SG * 4, P], bf16, name="wdvT")
            for vi in range(2):
                nc.sync.dma_start_transpose(
                    wdvT[:, vi], wdvB[:, vi].rearrange("p n d -> p (n d)"))
            st["wdvT"] = wdvT
            st["a_sb"] = a_sb
            st["hc0"] = hc0
            state[s] = st

        def stage_proj(s):
            st = state[s]
            a_sb, hc0, wdvT = st["a_sb"], st["hc0"], st["wdvT"]
            hB = bstp.tile([P, 2, SG, D], bf16, name="hB")
            st6 = bstp.tile([P, 2, SG, 6], f32, name="st6")
            mv = bstp.tile([P, 2, SG, 2], f32, name="mv")
            std4 = bstp.tile([P, 2 * SG], f32, name="std4")
            rs4 = bstp.tile([P, 2 * SG], f32, name="rs4")
            nmrs4 = bstp.tile([P, 2 * SG], f32, name="nmrs4")
            tB = tpool.tile([P, 2, SG, D], bf16, name="tB")
            for n in range(SG):
                for vi in range(2):
                    b_ps = ps_b.tile([P, D], f32, name="b_ps", tag="b")
                    for c in range(4):
                        nc.tensor.matmul(b_ps[:], wdvT[:, vi, 4 * n + c, :],
                                         owT_sb[:, c, :],
                                         start=(c == 0), stop=(c == 3))
                    base = hc0 if vi == 0 else a_sb
                    nc.vector.tensor_add(hB[:, vi, n, :], base[:, n, :], b_ps[:])
                    nc.vector.bn_stats(st6[:, vi, n, :], hB[:, vi, n, :])
                    nc.vector.bn_aggr(mv[:, vi, n, :], st6[:, vi, n, :])
            # rs = 1/sqrt(var+eps); nmrs = -mean*rs
            nc.scalar.activation(std4[:], mv[:, :, :, 1].rearrange(
                "p a n -> p (a n)"), AF.Sqrt, bias=eps_p1[:])
            nc.vector.reciprocal(rs4[:], std4[:])
            nc.vector.tensor_mul(nmrs4[:], mv[:, :, :, 0].rearrange(
                "p a n -> p (a n)"), rs4[:])
            nc.vector.tensor_scalar(nmrs4[:], nmrs4[:], -1.0, None, op0=OP.mult)
            for vi in range(2):
                for n in range(SG):
                    i4 = vi * SG + n
                    nc.scalar.activation(tB[:, vi, n, :], hB[:, vi, n, :],
                                         AF.Identity, scale=rs4[:, i4:i4 + 1],
                                         bias=nmrs4[:, i4:i4 + 1])
            if not unit_lng or not zero_lnb:
                # blend operand becomes lng*t + lnb (gate uses folded weights)
                thB = tpool.tile([P, 2, SG, D], bf16, name="thB")
                for vi in range(2):
                    for n in range(SG):
                        cur = tB[:, vi, n, :]
                        if not unit_lng:
                            nc.vector.tensor_mul(thB[:, vi, n, :], cur, lng_bc[:])
                            cur = thB[:, vi, n, :]
                        if not zero_lnb:
                            nc.vector.tensor_add(thB[:, vi, n, :], cur, lnb_bc[:])
                st["tblend"] = thB
            else:
                st["tblend"] = tB
            tT = tpool.tile([P, 2, SG * 4, P], bf16, name="tT")
            for vi in range(2):
                nc.sync.dma_start_transpose(
                    tT[:, vi], tB[:, vi].rearrange("p n d -> p (n d)"))
            st["tT"] = tT

        def stage_gate(s):
            st = state.pop(s)
            tT, mc = st["tT"], st["mc"]
            tbl = st["tblend"]
            rows = ts(s, W2)
            gsig = gp.tile([P, SG, D], bf16, name="gsig")
            bcc = gp.tile([P, SG, D], bf16, name="bcc")
            bml = gp.tile([P, SG, D], bf16, name="bml")
            o1 = gp.tile([P, SG, D], bf16, name="o1")
            o2 = gp.tile([P, SG, D], bf16, name="o2")
            ofin = gp.tile([P, SG, D], f32, name="ofin")
            for n in range(SG):
                pg = ps_pg.tile([P, D], f32, name="pg", tag="pg")
                for c in range(4):
                    nc.tensor.matmul(pg[:], tT[:, 0, 4 * n + c, :],
                                     gwT_sb[:, c, :],
                                     start=(c == 0), stop=False)
                for c in range(4):
                    nc.tensor.matmul(pg[:], tT[:, 1, 4 * n + c, :],
                                     gwT_sb[:, 4 + c, :],
                                     start=False, stop=(c == 3))
                if not zero_gb2:
                    nc.vector.tensor_add(pg[:], pg[:], gb2_bc[:])
                nc.scalar.activation(gsig[:, n, :], pg[:], AF.Sigmoid)
                nc.vector.tensor_scalar(bcc[:, n, :], gsig[:, n, :],
                                        mc[:, n, 2:3], mc[:, n, 4:5],
                                        op0=OP.mult, op1=OP.add)
                nc.vector.tensor_scalar(bml[:, n, :], gsig[:, n, :],
                                        mc[:, n, 3:4], mc[:, n, 5:6],
                                        op0=OP.mult, op1=OP.add)
            nc.gpsimd.tensor_mul(o1[:], bcc[:], tbl[:, 0])
            nc.gpsimd.tensor_mul(o2[:], bml[:], tbl[:, 1])
            nc.gpsimd.tensor_add(ofin[:], o1[:], o2[:])
            nc.sync.dma_start(
                out_d[rows, :].rearrange("(n p) d -> p n d", p=P), ofin[:])

        for s in range(nsg + 2):
            if s < nsg:
                stage_front(s)
            if 1 <= s <= nsg:
                stage_proj(s - 1)
            if 2 <= s <= nsg + 1:
                stage_gate(s - 2)

    nc.compile()
    return nc


def _get_nc(flags, bs=BS):
    key = (flags, bs)
    if key not in _cache:
        _cache[key] = _build(flags, bs)
    return _cache[key]


def kernel(x_cc, x_mlo, view_mask, in_proj_w, in_proj_b, out_w, out_b,
           ln_g, ln_b, gate_w, gate_b):
    from concourse import bass_utils

    x_cc = np.asarray(x_cc, np.float32)
    x_mlo = np.asarray(x_mlo, np.float32)
    view_mask = np.asarray(view_mask, np.float32)
    in_proj_w = np.asarray(in_proj_w, np.float32)
    in_proj_b = np.asarray(in_proj_b, np.float32)
    out_w = np.asarray(out_w, np.float32)
    out_b = np.asarray(out_b, np.float32)
    ln_g = np.asarray(ln_g, np.float32)
    ln_b = np.asarray(ln_b, np.float32)
    gate_w = np.asarray(gate_w, np.float32)
    gate_b = np.asarray(gate_b, np.float32)

    # ---- host-side weight prep ----
    Wq, Wk, Wv = in_proj_w[:D], in_proj_w[D:2 * D], in_proj_w[2 * D:]
    bq, bv = in_proj_b[:D], in_proj_b[2 * D:]
    Wvo = Wv.T @ out_w.T                                   # v_mlo @ Wo.T fold
    wvoiT = np.ascontiguousarray(Wvo + np.eye(D, dtype=np.float32))
    hb = bv @ out_w.T + out_b                              # const h bias
    lng2 = np.concatenate([ln_g, ln_g])
    lnb2 = np.concatenate([ln_b, ln_b])
    gate_w_f = gate_w * lng2[None, :]
    gb2 = gate_b + gate_w @ lnb2

    wqk8 = np.ascontiguousarray(
        np.concatenate([Wq.T, Wk.T], axis=1)).astype(FP8)
    wvT = np.ascontiguousarray(Wv.T).astype(BF16)
    wvoiT = wvoiT.astype(BF16)
    owT = np.ascontiguousarray(out_w.T).astype(BF16)
    gwT = np.ascontiguousarray(gate_w_f.T).astype(BF16)

    # ---- host-side input prep ----
    dx = x_cc - x_mlo
    a = view_mask[:, 0]
    m = view_mask[:, 1]
    bm = (m != 0).astype(np.float32)
    cf = ((a * m) > 0.5).astype(np.float32)
    u = 1.0 - cf
    mc = np.stack([bm, 1.0 - bm, cf, -cf, u * a, m * u + cf],
                  axis=1).astype(np.float32)

    flags = (
        not bq.any(),
        not hb.any(),
        not gb2.any(),
        bool((ln_g == 1.0).all()),
        not ln_b.any(),
    )
    nc = _get_nc(flags)

    in_maps = []
    for c in range(NCORES):
        sl = slice(c * BS, (c + 1) * BS)
        xc, xm, dxc = x_cc[sl], x_mlo[sl], dx[sl]
        xcT = np.ascontiguousarray(xc.T)
        xmT = np.ascontiguousarray(xm.T)
        dxT = np.ascontiguousarray(dxc.T)
        mm = {
            "xT8c": xcT.astype(FP8), "xT8m": xmT.astype(FP8),
            "dxT8": dxT.astype(FP8), "xTm": xmT.astype(BF16),
            "dxT": dxT.astype(BF16), "dxF": dxc.astype(BF16),
            "mc": mc[sl],
            "wqk8": wqk8, "wvT": wvT, "wvoiT": wvoiT, "owT": owT, "gwT": gwT,
        }
        zero_qb, zero_hb, zero_gb2, unit_lng, zero_lnb = flags
        if not zero_qb:
            mm["qb2"] = bq
        if not zero_hb:
            mm["hb"] = hb
        if not zero_gb2:
            mm["gb2"] = gb2
        if not unit_lng:
            mm["lng"] = ln_g
        if not zero_lnb:
            mm["lnb"] = ln_b
        in_maps.append(mm)

    global _last_run
    _last_run = (nc, in_maps)
    res = bass_utils.run_bass_kernel_spmd(nc, in_maps, core_ids=list(range(NCORES)))
    return np.concatenate([r["out"] for r in res.results], axis=0)
